# revision 24
# baseline (speedup 1.0000x reference)
"""Self-contained Trainium2 Bass kernel for nn_GCNResnet (batch-attention GCN).

Math (reference collapse):
  out[b,:] = sum_n c_n * softmax(X_n X_n^T)[b,:] @ (X_n @ W) + offset*(1_D @ W)
where X_n = x[:, n, :] ([B=4096, D=10]); c_n and offset fold BN(eval) +
adjacency + GCN + avgpool scalars. Per node the softmax normalizer is folded
into the PV matmul via a ones column:
  U_n = exp(X_n X_n^T) @ [c_n*(X_n@W) | 1]   -> out_n = U[:, :10] / U[:, 10]
(no max-subtraction needed: |scores| <= ~45 << 88, exp stays in fp32 range).

Sharding: row-slab parallel over 8 cores, 512 query rows per core, no
collectives. The final divide+transpose+partial-sum happens on host during
the gather/unshard step.

Perf structure (PE measured at 1.2 GHz steady on this part; 32-row/col
tiled matmuls at different tile_positions stream concurrently, ~4x):
  - Scores: chunk ck on strip ck%4 (xt replicated per strip host-side), a
    pair of 2-chunk groups emits one 4-wide row-tiled quad; 3-deep PSUM ring
    double-buffers ahead of the exps.
  - PV: chunks map to the 4 column-tile positions; all 4 partial-U slices
    live in ONE psum bank (partition-sliced), and a pair's 4 PV matmuls run
    as one concurrent col-tiled quad. PV trails the scores by 3 pairs so its
    exps are always long done. U double-buffers across nodes (2 banks).
  - exp alternates between the scalar engine (native Exp, bf16 out) and the
    vector engine (Schraudolph bit-trick: int16 round(A*s+B) bitcast to
    bf16; the ~3%% sawtooth error cancels between softmax numerator and
    denominator; end-to-end rel err ~4e-3, gate is 2e-2).
  - float32r/bf16 DRAM params (numpy feeds raw fp32/bf16), no casting DMAs,
    xh pre-arranged host-side into its SBUF layout -> few fat descriptors;
    node-0 slices of all strips + node-0 xh land first so compute starts
    ~3us after the fixed ~7.5us framework preamble.
  - ACT exp table preloaded during the input-DMA window; teardown skips the
    per-semaphore clears (entry init value-SETs them anyway).
"""

import sys

if "/opt/trn_rl_repo" not in sys.path:
    sys.path.insert(0, "/opt/trn_rl_repo")

import ml_dtypes
import numpy as np

import concourse.bass as bass
import concourse.mybir as mybir
from concourse import tile
from concourse.bass_utils import run_bass_kernel_spmd
from concourse.vector_clock import ScopedClock

B, N, D = 4096, 3, 10
NCORES = 8
R = B // NCORES            # 512 query rows per core
KC = B // 128              # 32 key chunks of 128
BN_EPS = 1e-5

NS = 4                     # row-tile strips
SW = KC // NS              # key chunks per strip (8)
SCOLS = SW * 128 + R       # strip columns per node: 8 chunks + query slab
GW = 2                     # chunks per exp group
NG = KC // GW              # groups per node (16)

# Schraudolph exp constants (bf16): exp(s) ~= bitcast16(round(A*s + SB))
SCH_A = float((1 << 7) / np.log(2.0))
SCH_B = float(127.0 * (1 << 7) - 5.72)

EXP_DVE = True             # odd groups use the DVE Schraudolph path


def _patched_drain_and_barrier(self, tick_clock, wait_clock):
    # Walrus in this container rejects >1 sync-wait on a CTRL-class
    # instruction; absorb the tail-drain waits into SP nops, one wait each.
    nc = self.nc
    probe = nc.sync.nop()
    wait_clock.add_sem_waits(probe.ins, ScopedClock({None: tick_clock.global_clock}))
    si = probe.ins.sync_info
    waits = list(si.on_wait) if si is not None else []
    upds = list(si.on_update) if si is not None else []
    probe.ins.sync_info = mybir.SyncInfo(on_wait=waits[:1], on_update=upds)
    for w in waits[1:]:
        n = nc.sync.nop()
        n.ins.sync_info = mybir.SyncInfo(on_wait=[w], on_update=[])
    nc.sync.drain()
    nc.all_engine_barrier()
    assert self.sems is not None
    popped = nc._tile_sem_poison_stack.pop()
    assert popped is self._sem_poison
    # Skip the per-sem clear + second barrier: the program value-SETs its
    # semaphores during entry init, so a fresh execution never observes the
    # previous run's end-state. Bookkeeping only (no instructions).
    sems = list(self.sems.allocated().values())
    sem_nums = [s.num if hasattr(s, "num") else s for s in sems]
    nc._state.prepend_free_semaphores(sem_nums)
    for poison_set in nc._tile_sem_poison_stack:
        poison_set.update(sem_nums)


tile.TileContext._drain_and_barrier = _patched_drain_and_barrier

_MAX_WAITS = 1
_waitsplit_ctr = [0]


def _split_sync_waits(nc):
    """Walrus here allows very few sync-waits per instruction. Move excess
    waits onto same-engine no-ops placed immediately before the instruction
    (engine streams are in-order, so semantics are preserved)."""
    for f in nc.m.functions:
        for bb in f.blocks:
            new = []
            changed = False
            for inst in bb.instructions:
                si = inst.sync_info
                waits = list(si.on_wait) if si is not None else []
                if len(waits) > _MAX_WAITS:
                    changed = True
                    for w in waits[:-_MAX_WAITS]:
                        _waitsplit_ctr[0] += 1
                        nop = mybir.InstNoOp(
                            name=f"I-waitsplit-{_waitsplit_ctr[0]}", ins=[], outs=[]
                        )
                        nop.engine = inst.engine
                        nop.sync_info = mybir.SyncInfo(on_wait=[w], on_update=[])
                        new.append(nop)
                    inst.sync_info = mybir.SyncInfo(
                        on_wait=waits[-_MAX_WAITS:], on_update=list(si.on_update)
                    )
                new.append(inst)
            if changed:
                bb.instructions = new


def build_nc(rep: int = 1, rep_marker: bool = False, mode: str = "full") -> bass.Bass:
    """One-core SPMD program: full keys replicated, this core's 512-row slab.

    mode: "full" (loads+compute per rep), "loads" (DMAs only per rep),
    "compute" (loads once, compute per rep) — for timing decomposition.
    """
    f32 = mybir.dt.float32
    f32r = mybir.dt.float32r
    bf16 = mybir.dt.bfloat16
    i16 = mybir.dt.int16
    nc = bass.Bass()

    # Strip layout: strip s holds key chunks {ck : ck%4 == s} (slot ck//4)
    # plus this core's query slab, for all 3 nodes: [10, N*SCOLS] contiguous.
    xts = nc.declare_dram_parameter("xts", [NS, D, N * SCOLS], f32r, isOutput=False)
    # PV weights pre-arranged host-side into the SBUF layout [128, N*KC*11]:
    # xh[p, (n*KC+ck)*11 + d] = (c_n * X_n W | 1)[ck*128+p, d]
    xh = nc.declare_dram_parameter("xh", [128, N * KC * (D + 1)], bf16, isOutput=False)
    # col-tiled partial-U: partial c of node n at rows 32c..32c+10,
    # cols 512n..512(n+1); host sums the 4 partials + normalizes
    uout = nc.declare_dram_parameter("uout", [128, N * 512 + 4], f32, isOutput=True)

    with tile.TileContext(nc) as tc:
        with (
            tc.tile_pool(name="xtp", bufs=1) as xtp,
            tc.tile_pool(name="xhp", bufs=1) as xhp,
            tc.tile_pool(name="etp", bufs=8) as etp,
            tc.tile_pool(name="aux", bufs=1) as auxp,
            tc.tile_pool(name="scr", bufs=3, space="PSUM") as pssR,
            tc.tile_pool(name="psu", bufs=2, space="PSUM") as psu,
        ):
            xt_sb = xh_sb = None
            warmed = False
            for rep_i in range(rep):
                if mode != "compute" or rep_i == 0:
                    xt_sb = xtp.tile([128, N * SCOLS], f32r, tag="xt", name="xt")
                    # node-0 slices of all 4 strips land first (first pair of
                    # score quads needs them); remainder + xh stream behind
                    nc.sync.dma_start(xt_sb[0:D, 0:SCOLS], xts[0][:, 0:SCOLS])
                    nc.scalar.dma_start(
                        xt_sb[32 : 32 + D, 0:SCOLS], xts[1][:, 0:SCOLS]
                    )
                    nc.gpsimd.dma_start(
                        xt_sb[64 : 64 + D, 0:SCOLS], xts[2][:, 0:SCOLS]
                    )
                    nc.sync.dma_start(
                        xt_sb[96 : 96 + D, 0:SCOLS], xts[3][:, 0:SCOLS]
                    )
                    xh_sb = xhp.tile([128, N * KC * (D + 1)], bf16, tag="xh")
                    XW = KC * (D + 1)
                    nc.scalar.dma_start(xh_sb[:, 0:XW], xh[:, 0:XW])
                    nc.scalar.dma_start(xh_sb[:, XW:], xh[:, XW:])
                    nc.sync.dma_start(xt_sb[0:D, SCOLS:], xts[0][:, SCOLS:])
                    nc.gpsimd.dma_start(xt_sb[32 : 32 + D, SCOLS:], xts[1][:, SCOLS:])
                    nc.gpsimd.dma_start(xt_sb[64 : 64 + D, SCOLS:], xts[2][:, SCOLS:])
                    nc.gpsimd.dma_start(xt_sb[96 : 96 + D, SCOLS:], xts[3][:, SCOLS:])
                if not warmed:
                    # Warm the ACT exp table while input DMAs are in flight.
                    warmed = True
                    warm_in = auxp.tile([1, 8], f32, tag="warm_in")
                    warm_out = auxp.tile([1, 8], f32, tag="warm_out")
                    nc.vector.memset(warm_in[:], 0.0)
                    nc.scalar.activation(
                        warm_out[:], warm_in[:], mybir.ActivationFunctionType.Exp
                    )
                if mode == "loads":
                    continue

                # flat pipeline over all (node, group) pairs; the per-node
                # U accumulator double-buffers (2 PSUM banks) so the pipeline
                # never drains at node boundaries
                u_tiles = {}
                ets = {}

                def emit_pv(G):
                    n, g = divmod(G, NG)
                    u_ps = u_tiles[n]
                    et = ets.pop(G)
                    for i in range(GW):
                        ck = GW * g + i
                        c = ck % NS
                        nc.tensor.matmul(
                            u_ps[32 * c : 32 * c + D + 1, :],
                            lhsT=xh_sb[
                                :,
                                (n * KC + ck) * (D + 1) : (n * KC + ck + 1)
                                * (D + 1),
                            ],
                            rhs=et[:, 512 * i : 512 * (i + 1)],
                            tile_position=(0, 32 * c),
                            start=(ck < NS),
                            stop=(ck >= KC - NS),
                        )

                def emit_sc_exp(G):
                    n, g = divmod(G, NG)
                    if g == 0:
                        u_tiles[n] = psu.tile([128, 512], f32, tag="u", name="u_ps")
                    ps = pssR.tile([128, 512 * GW], f32, tag="s", name="scr")
                    for i in range(GW):
                        ck = GW * g + i
                        s, j = ck % NS, ck // NS
                        base = n * SCOLS
                        nc.tensor.matmul(
                            ps[:, 512 * i : 512 * (i + 1)],
                            lhsT=xt_sb[
                                32 * s : 32 * s + D,
                                base + 128 * j : base + 128 * (j + 1),
                            ],
                            rhs=xt_sb[
                                32 * s : 32 * s + D,
                                base + SW * 128 : base + SCOLS,
                            ],
                            tile_position=(32 * s, 0),
                        )
                    et = etp.tile([128, 512 * GW], bf16, tag="et")
                    if EXP_DVE and g % 2:
                        # Schraudolph exp on DVE: int16 mult-add, bitcast
                        nc.vector.tensor_scalar(
                            et[:].bitcast(i16),
                            ps[:],
                            SCH_A,
                            SCH_B,
                            mybir.AluOpType.mult,
                            mybir.AluOpType.add,
                        )
                    else:
                        nc.scalar.activation(
                            et[:], ps[:], mybir.ActivationFunctionType.Exp
                        )
                    ets[G] = et

                def emit_evac(n):
                    # evacuate U (4 col-sliced partials live in one bank)
                    u_sb = etp.tile(
                        [128, 512], f32, tag="usb", bufs=2, name="u_sb"
                    )
                    u_ps = u_tiles.pop(n)
                    if n % 2:
                        nc.vector.tensor_copy(u_sb[:], u_ps[:])
                    else:
                        nc.scalar.activation(
                            u_sb[:], u_ps[:], mybir.ActivationFunctionType.Copy
                        )
                    nc.sync.dma_start(uout[:, 512 * n : 512 * (n + 1)], u_sb[:])

                # pair-granularity software pipeline: both score groups of a
                # pair back-to-back (4 strips -> one 4-wide quad), then PV
                # from TWO pairs back (its exps long done -> the pair's 4 PV
                # matmuls fuse into one concurrent 4-wide col-group quad) ->
                # 2 tiling-mode switches per 4 chunks.
                NPAIR = N * NG // 2
                for p in range(NPAIR + 3):
                    if p < NPAIR:
                        emit_sc_exp(2 * p)
                        emit_sc_exp(2 * p + 1)
                        if mode == "nopv":
                            ets.clear()
                            continue
                    if mode == "nopv":
                        continue
                    if p >= 3:
                        emit_pv(2 * p - 6)
                        emit_pv(2 * p - 5)
                        if (2 * p - 5) % NG == NG - 1:
                            emit_evac((2 * p - 5) // NG)
                if rep_marker and mode != "nopv":
                    mark = auxp.tile([1, 4], f32, tag="mark")
                    nc.vector.memset(mark[:], float(rep_i))
                    nc.sync.dma_start(uout[0:1, N * 512 : N * 512 + 4], mark[:])
    _split_sync_waits(nc)
    return nc


def _host_prep(x, A, gc_weight, bn_gamma, bn_beta, bn_mean, bn_var):
    x = np.asarray(x, np.float32)
    A = np.asarray(A, np.float32)
    W = np.asarray(gc_weight, np.float32)
    scale = np.asarray(bn_gamma, np.float32) / np.sqrt(
        np.asarray(bn_var, np.float32) + BN_EPS
    )
    d_half = 0.5 * np.eye(N, dtype=np.float32)
    a0 = np.ones((N, N), np.float32) - np.eye(N, dtype=np.float32)
    adj = d_half @ (a0 + A) @ d_half
    wk = 0.5 * (adj[0] + adj[1])                      # [N]
    cn = (wk * scale).astype(np.float32)              # [N]
    offset = float(
        np.sum(wk * (np.asarray(bn_beta, np.float32)
                     - np.asarray(bn_mean, np.float32) * scale))
    )
    bias_vec = (offset * W.sum(axis=0)).astype(np.float32)  # [D]

    # keys part of the strip layout: [NS, D, N, SCOLS] (query slab zeroed,
    # filled per-core in _in_maps)
    xk = x.transpose(1, 2, 0).reshape(N, D, KC, 128)  # [n, d, ck, 128]
    xts = np.zeros((NS, D, N, SCOLS), np.float32)
    for s in range(NS):
        # chunks ck = s, s+NS, ... at slots 0..SW-1
        xts[s, :, :, : SW * 128] = (
            xk[:, :, s::NS, :].transpose(1, 0, 2, 3).reshape(D, N, SW * 128)
        )

    xh = np.empty((N, KC, 128, D + 1), np.float32)
    for n in range(N):
        yh = np.empty((B, D + 1), np.float32)
        yh[:, :D] = (x[:, n, :] @ W) * cn[n]
        yh[:, D] = 1.0
        xh[n] = yh.reshape(KC, 128, D + 1)
    xh = np.ascontiguousarray(
        xh.transpose(2, 0, 1, 3).reshape(128, N * KC * (D + 1))
    ).astype(ml_dtypes.bfloat16)
    xq = x.transpose(1, 2, 0)                         # [N, D, B]
    return (xts, xq), xh, bias_vec


def _in_maps(xt, xh):
    xts, xq = xt
    maps = []
    for c in range(NCORES):
        xc = xts.copy()
        for s in range(NS):
            xc[s, :, :, SW * 128 :] = xq[:, :, c * R : (c + 1) * R].transpose(1, 0, 2)
        maps.append(
            {"xts": xc.reshape(NS, D, N * SCOLS), "xh": xh}
        )
    return maps


def _finish(uouts, bias_vec):
    """Host gather: sum the 4 partial-U slabs, normalize (divide by the
    folded rowsum), transpose to [rows, D], sum nodes, concatenate core
    slabs, add the BN/adjacency bias."""
    out = np.empty((B, D), np.float32)
    for c in range(NCORES):
        u = uouts[c]                                   # [128, N*512 + 4]
        acc = np.zeros((512, D), np.float32)
        for n in range(N):
            blk = u[:, 512 * n : 512 * (n + 1)]
            un = sum(blk[32 * q : 32 * q + D + 1] for q in range(NS))
            acc += (un[:D] / un[D]).T
        out[c * R : (c + 1) * R] = acc
    return out + bias_vec[None, :]


def kernel(**inputs) -> np.ndarray:
    assert inputs["x"].shape == (B, N, D)
    xt, xh, bias_vec = _host_prep(**inputs)
    nc = build_nc(rep=1)
    res = run_bass_kernel_spmd(nc, _in_maps(xt, xh), list(range(NCORES)))
    return _finish(
        [res.results[c]["uout"] for c in range(NCORES)], bias_vec
    ).astype(np.float32)
